# revision 7
# baseline (speedup 1.0000x reference)
"""Trainium2 Bass kernel for nn_Codec (autoregressive raster-scan codec).

Wavefront decomposition: pixel (ky,kx) of the 122x122 delta grid is computed
at step t = 4*ky + kx (skew-4 anti-diagonal), a 606-step serial chain with 8
cores x 3 images each (data-parallel over the 24 (b,c) pairs); 96 lanes per
core = 3 images x 32 row-slots (slot = ky mod 32).

v2 redesign (vs the shift-DMA baseline):
  - fp16 operands on the PE (1 cycle/row; fp32 runs 2 passes at half rate and
    doubles the LDWEIGHTS+MATMUL instruction count).
  - The kernel stores e(t) = clip(z7*mask) (the clipped prediction) in a
    32-row SBUF ring (row t%32). Delta features dm = x*gridmask - e split:
    the x part is host-precomputed into the feature stream (24 extra rows);
    the e part enters via 4 ring matmuls (one per row-shift q=0..3, lane
    shifts via rhs/out free-dim offset APs, +3 wrap matmuls) using
    phase-packed negated weights (32 phases, row r of phase p holds the
    weight for e(t-k), k=(p-r) mod 32).
  - z1 (48) and the residual z5 preload (24) live in ONE 72-partition PSUM
    tile, so every preload matmul feeds both in one instruction.
  - b7 enters via a constant ones row appended to h6 (13-row rhs).
  - Tail is 2 DVE ops: t0 = z7*mask; e = clip(t0) written into the ring.
    The final delta dm = x_center - e is computed on the HOST (it has x).
  - No gpsimd DMAs at all; every 16 steps the freshly-written ring half is
    copied to a staging tile (DVE) and DMA'd to DRAM.
"""
import sys

sys.path.insert(0, "/opt/trn_rl_repo")
import numpy as np

R = 3
DH = DW = 122
NSTEP = 4 * (DH - 1) + DW  # 606
NL = 96                    # lanes per core = 3 images x 32 slots
CH = 32                    # steps per x-feature chunk
NCHUNK = (NSTEP + CH - 1) // CH          # 19
TPAD = NCHUNK * CH                       # 608
NBLK = (NSTEP + 15) // 16                # 38 output blocks of 16 steps

# (q, d) pairs for the 24 delta features, with W1/W5 column index.
# features 24..30: dy=3 (q=3), dx=-3..3 ; 31..37: q=2 ; 38..44: q=1 ;
# 45..47: left3 = q=0, d=-3..-1
QD = []
for q in (3, 2, 1):
    for d in range(-3, 4):
        QD.append((q, d, 24 + (3 - q) * 7 + (d + 3)))
for d in (-3, -2, -1):
    QD.append((0, d, 48 + d))
assert len(QD) == 24 and all(24 <= c < 48 for (_, _, c) in QD)

_TRACE = False
_TRACE_KW = {}
_LAST_RESULTS = None

# ---------------------------------------------------------------- consts layout
_C16 = {}
_cc16 = 0


def _span16(name, rows, cols):
    global _cc16
    _C16[name] = (rows, _cc16, cols)
    _cc16 += cols


_span16("wx", 48, 88)
for _q in range(4):
    for _p in range(32):
        _span16(f"wd{_q}_{_p}", 32, 88)
_span16("wf47", 1, 88)
_span16("wf44", 1, 88)
_span16("w2T", 48, 48)
_span16("w3T", 48, 48)
_span16("w4T", 48, 48)
_span16("w5T", 48, 24)
_span16("w6T", 24, 12)
_span16("w7b", 13, 1)
CC16 = _cc16

_C32 = {}
_cc32 = 0


def _span32(name, rows, cols):
    global _cc32
    _C32[name] = (rows, _cc32, cols)
    _cc32 += cols


for _i in range(1, 7):
    _span32(f"b{_i}", 48, 1)
CC32 = _cc32


def _pack_consts(W):
    """Returns (c16 (48, CC16) float16, c32 (48, CC32) float32)."""
    c16 = np.zeros((48, CC16), np.float16)
    c32 = np.zeros((48, CC32), np.float32)

    def put16(name, arr):
        rows, c0, cols = _C16[name]
        assert arr.shape == (rows, cols), (name, arr.shape)
        c16[:rows, c0:c0 + cols] = arr.astype(np.float16)

    W1, W5 = W["W1"], W["W5"]

    # mm_x stationary (48K x 72M): rows 0-23 raw x_nb -> [W1x | W5x],
    # rows 24-47 xm_nb (x*gridmask at QD offsets) -> [+W1 qd col | +W5 qd col]
    wx = np.zeros((48, 88), np.float32)
    wx[0:24, 0:48] = W1[:, 0:24].T
    wx[0:24, 64:88] = W5[:, 0:24].T
    for j, (q, d, col) in enumerate(QD):
        wx[24 + j, 0:48] = W1[:, col]
        wx[24 + j, 64:88] = W5[:, col]
    put16("wx", wx)

    # ring phase weights: ring row r at step t (phase p=t%32) holds e(t-k),
    # k = (p - r) mod 32.  Contribution of e(t-k) with shift q: feature
    # (q, d=4q-k) -> weight = -W1[:, col], -W5[:, col].
    valid_k = {0: range(2, 4), 1: range(2, 8), 2: range(5, 12), 3: range(9, 16)}
    colof = {(q, d): c for (q, d, c) in QD}
    for q in range(4):
        for p in range(32):
            wd = np.zeros((32, 88), np.float32)
            for r_ in range(32):
                k = (p - r_) % 32
                if k in valid_k[q]:
                    col = colof[(q, 4 * q - k)]
                    wd[r_, 0:48] = -W1[:, col]
                    wd[r_, 64:88] = -W5[:, col]
            put16(f"wd{q}_{p}", wd)

    wf47 = np.zeros((1, 88), np.float32)
    wf47[0, 0:48] = -W1[:, 47]
    wf47[0, 64:88] = -W5[:, 47]
    put16("wf47", wf47)
    wf44 = np.zeros((1, 88), np.float32)
    wf44[0, 0:48] = -W1[:, 44]
    wf44[0, 64:88] = -W5[:, 44]
    put16("wf44", wf44)
    put16("w2T", W["W2"].T)
    put16("w3T", W["W3"].T)
    put16("w4T", W["W4"].T)
    put16("w5T", W5.T)
    put16("w6T", W["W6"].T)
    put16("w7b", np.concatenate([W["W7"].T, W["b7"][None, :]], axis=0))

    def put32(name, arr):
        rows, c0, cols = _C32[name]
        assert arr.shape == (rows, cols), (name, arr.shape)
        c32[:rows, c0:c0 + cols] = arr.astype(np.float32)

    for i in range(1, 7):
        b = W[f"b{i}"]
        put32(f"b{i}", np.pad(b[:, None], ((0, 48 - b.shape[0]), (0, 0))))
    return c16, c32


def _build_xfeat(xcore):
    """xcore (3,128,128) -> xf16 (48, TPAD*96) fp16, mk32 (1, TPAD*96) fp32.

    Rows 0-23: raw x neighborhood (matches W1[:, :24] feature order).
    Rows 24-47: x*gridmask at the QD (q,d) offsets (the +x part of the
    delta features; the -e part comes from the on-device ring).
    """
    xf = np.zeros((48, TPAD, NL), np.float16)
    mk = np.zeros((1, TPAD, NL), np.float32)
    ky, kx = np.meshgrid(np.arange(DH), np.arange(DW), indexing="ij")
    tf = (4 * ky + kx).ravel()
    F = np.empty((DH, DW, 48), np.float32)
    for g in range(3):
        img = xcore[g]
        col = (g * 32 + (ky % 32)).ravel()
        for i in range(3):
            for j in range(7):
                F[:, :, 7 * i + j] = img[i:i + DH, j:j + DW]
        for j in range(3):
            F[:, :, 21 + j] = img[3:3 + DH, j:j + DW]
        for j, (q, d, _) in enumerate(QD):
            a, b = ky - q, kx + d
            v = np.zeros((DH, DW), np.float32)
            ok = (a >= 0) & (a < DH) & (b >= 0) & (b < DW)
            am, bm = np.clip(a, 0, DH - 1), np.clip(b, 0, DW - 1)
            v = np.where(ok, img[am + 3, bm + 3], 0.0)
            F[:, :, 24 + j] = v
        xf[:, tf, col] = F.reshape(-1, 48).T.astype(np.float16)
        mk[0, tf, col] = 1.0
    return xf.reshape(48, TPAD * NL), mk.reshape(1, TPAD * NL)


def _g3(ap):
    return ap.rearrange("p (g c) -> p g c", g=3)


def _build_program():
    import concourse.bass as bass  # noqa: F401
    from concourse import bacc
    import concourse.mybir as mybir
    from concourse.tile import TileContext

    F32 = mybir.dt.float32
    F16 = mybir.dt.float16
    AF = mybir.ActivationFunctionType
    OP = mybir.AluOpType

    nc = bacc.Bacc(trn_type="TRN2", num_devices=8)
    xfeat_d = nc.dram_tensor("xfeat", [48, TPAD * NL], F16, kind="ExternalInput")
    mask_d = nc.dram_tensor("maskf", [1, TPAD * NL], F32, kind="ExternalInput")
    c16_d = nc.dram_tensor("c16", [48, CC16], F16, kind="ExternalInput")
    c32_d = nc.dram_tensor("c32", [48, CC32], F32, kind="ExternalInput")
    estore_d = nc.dram_tensor("estore", [NBLK * 16, NL], F16, kind="ExternalOutput")

    with TileContext(nc) as tc:
        with tc.tile_pool(name="wp", bufs=1) as wp, \
             tc.tile_pool(name="chp", bufs=3) as chp, \
             tc.tile_pool(name="mkp", bufs=3) as mkp, \
             tc.tile_pool(name="rp", bufs=1) as rp, \
             tc.tile_pool(name="ep", bufs=2) as ep, \
             tc.tile_pool(name="hp", bufs=2) as hp, \
             tc.tile_pool(name="h6p", bufs=1) as h6p, \
             tc.tile_pool(name="tp", bufs=2) as tp, \
             tc.tile_pool(name="z15p", bufs=2, space="PSUM") as z15p, \
             tc.tile_pool(name="zp", bufs=1, space="PSUM") as zp:

            ct16 = wp.tile([48, CC16], F16)
            ct32 = wp.tile([48, CC32], F32)
            nc.sync.dma_start(out=ct16, in_=c16_d[:, :])
            nc.sync.dma_start(out=ct32, in_=c32_d[:, :])

            def cs16(name, r0=0, rows=None):
                r, c0, cols = _C16[name]
                rr = r if rows is None else rows
                return ct16[r0:r0 + rr, c0:c0 + cols]

            def cs32(name, r0=0, rows=None):
                r, c0, cols = _C32[name]
                rr = r if rows is None else rows
                return ct32[r0:r0 + rr, c0:c0 + cols]

            # e ring: row t%32 holds e(t) = clip(z7*mask), fp16.
            # Written via per-step DMA from the (1,96) e tile (compute
            # engines cannot address partition offsets other than 0/32/64/96).
            ring = rp.tile([32, NL], F16, tag="ring")
            nc.vector.memset(ring[:, :], 0.0)

            # h6x: rows 0-11 = h6 (ACT), row 12 = const ones (b7 injection)
            h6x = h6p.tile([13, NL], F16, tag="h6x")
            nc.vector.memset(h6x[:, :], 1.0)

            chunks = {}

            def get_chunk(c):
                if c not in chunks and c < NCHUNK:
                    ch_t = chp.tile([48, CH * NL], F16, name="ch", tag="ch")
                    mk_t = mkp.tile([1, CH * NL], F32, name="mk", tag="mk")
                    lo, hi = c * CH * NL, (c + 1) * CH * NL
                    nc.sync.dma_start(out=ch_t, in_=xfeat_d[:, lo:hi])
                    nc.sync.dma_start(out=mk_t, in_=mask_d[:, lo:hi])
                    chunks[c] = (ch_t, mk_t)
                return chunks.get(c)

            for t in range(NSTEP):
                c = t // CH
                ch_t, mk_t = get_chunk(c)
                if t % CH == 0:
                    get_chunk(c + 1)  # prefetch next chunk
                off = (t - c * CH) * NL
                xs = ch_t[0:48, off:off + NL]
                maskr = mk_t[0:1, off:off + NL]
                p = t % 32

                ringg = _g3(ring[:, :])

                # ---------------- z1(48)+z5(24) merged PSUM preload ----------
                z15 = z15p.tile([88, NL], F32, tag="z15")
                z15g = _g3(z15[:, :])
                nc.tensor.matmul(z15[:, :], cs16("wx"), xs, start=True,
                                 stop=False)
                # ring matmuls, shift q = dest lane s <- ring lane s-q
                # q=3, q=2 first (their fresh rows have >=5 steps of slack)
                for q in (3, 2):
                    nc.tensor.matmul(z15g[:, :, q:32], cs16(f"wd{q}_{p}"),
                                     ringg[:, :, 0:32 - q],
                                     start=False, stop=False)
                    nc.tensor.matmul(z15g[:, :, 0:q], cs16(f"wd{q}_{p}"),
                                     ringg[:, :, 32 - q:32],
                                     start=False, stop=False)
                nc.tensor.matmul(z15[:, :], cs16(f"wd0_{p}"), ring[:, :],
                                 start=False, stop=False)
                nc.tensor.matmul(z15g[:, :, 1:32], cs16(f"wd1_{p}"),
                                 ringg[:, :, 0:31], start=False, stop=False)
                nc.tensor.matmul(z15g[:, :, 0:1], cs16(f"wd1_{p}"),
                                 ringg[:, :, 31:32], start=False,
                                 stop=(t == 0))
                # fresh k=1 contributions from e(t-1): feature 47 (q=0) and
                # feature 44 (q=1, lane shift 1 + wrap)
                if t > 0:
                    epg = _g3(eprev[:, :])
                    nc.tensor.matmul(z15[:, :], cs16("wf47"), eprev[:, :],
                                     start=False, stop=False)
                    nc.tensor.matmul(z15g[:, :, 1:32], cs16("wf44"),
                                     epg[:, :, 0:31], start=False, stop=False)
                    nc.tensor.matmul(z15g[:, :, 0:1], cs16("wf44"),
                                     epg[:, :, 31:32], start=False, stop=True)

                # ---------------- MLP chain ----------------
                h1 = hp.tile([48, NL], F16, tag="h1")
                nc.scalar.activation(h1[:, :], z15[0:48, :], AF.Lrelu,
                                     bias=cs32("b1"), scale=1.0, alpha=0.01)
                z2 = zp.tile([48, NL], F32, tag="z2")
                nc.tensor.matmul(z2[:, :], cs16("w2T"), h1[:, :], start=True,
                                 stop=True)
                h2 = hp.tile([48, NL], F16, tag="h2")
                nc.scalar.activation(h2[:, :], z2[:, :], AF.Lrelu,
                                     bias=cs32("b2"), scale=1.0, alpha=0.01)
                z3 = zp.tile([48, NL], F32, tag="z3")
                nc.tensor.matmul(z3[:, :], cs16("w3T"), h2[:, :], start=True,
                                 stop=True)
                h3 = hp.tile([48, NL], F16, tag="h3")
                nc.scalar.activation(h3[:, :], z3[:, :], AF.Lrelu,
                                     bias=cs32("b3"), scale=1.0, alpha=0.01)
                z4 = zp.tile([48, NL], F32, tag="z4")
                nc.tensor.matmul(z4[:, :], cs16("w4T"), h3[:, :], start=True,
                                 stop=True)
                h4 = hp.tile([48, NL], F16, tag="h4")
                nc.scalar.activation(h4[:, :], z4[:, :], AF.Lrelu,
                                     bias=cs32("b4"), scale=1.0, alpha=0.01)
                nc.tensor.matmul(z15[64:88, :], cs16("w5T"), h4[:, :],
                                 start=False, stop=True)
                h5 = hp.tile([24, NL], F16, tag="h5")
                nc.scalar.activation(h5[:, :], z15[64:88, :], AF.Lrelu,
                                     bias=cs32("b5", rows=24), scale=1.0,
                                     alpha=0.01)
                z6 = zp.tile([12, NL], F32, tag="z6")
                nc.tensor.matmul(z6[:, :], cs16("w6T"), h5[:, :], start=True,
                                 stop=True)
                nc.scalar.activation(h6x[0:12, :], z6[:, :], AF.Lrelu,
                                     bias=cs32("b6", rows=12), scale=1.0,
                                     alpha=0.01)
                z7 = zp.tile([1, NL], F32, tag="z7")
                nc.tensor.matmul(z7[:, :], cs16("w7b"), h6x[:, :], start=True,
                                 stop=True)

                # ---------------- tail: e = clip(z7*mask) -> ring row --------
                t0 = tp.tile([1, NL], F32, tag="t0")
                nc.vector.tensor_tensor(out=t0[:, :], in0=z7[:, :], in1=maskr,
                                        op=OP.mult)
                ecur = ep.tile([1, NL], F16, tag="e")
                nc.vector.tensor_scalar(out=ecur[:, :], in0=t0[:, :],
                                        scalar1=1.0, scalar2=-1.0,
                                        op0=OP.min, op1=OP.max)
                # background: e -> ring row (1 step of slack; ring serves k>=2)
                nc.sync.dma_start(out=ring[p:p + 1, :], in_=ecur[:, :])
                eprev = ecur

                # ---------------- output: DMA each finished 16-row half ------
                if t % 16 == 15 or t == NSTEP - 1:
                    blk = t // 16
                    r0 = (blk % 2) * 16
                    nc.sync.dma_start(out=estore_d[blk * 16:(blk + 1) * 16, :],
                                      in_=ring[r0:r0 + 16, :])

    nc.finalize()
    return nc


_PROGRAM = None


def _finalize_outputs(D_all):
    """D_all (8,3,122,122) float32 deltas -> (loss, invCR)."""
    b, ch, h, w = 8, 3, 128, 128
    deltas = np.zeros((b, ch, h - 2, w), np.float32)
    deltas[:, :, R:R + DH, R:R + DW] = D_all
    loss = np.sqrt(np.mean(np.square(deltas), dtype=np.float32), dtype=np.float32)
    de = deltas[:, :, R:, R:-R]
    hist, _ = np.histogram(de, bins=256, range=(-1.0, 1.0))
    prob = hist.astype(np.float32) / np.float32(de.size)
    logp = np.zeros_like(prob)
    np.log2(prob, out=logp, where=prob > 0)
    invCR = np.float32(np.sum(-prob * logp, dtype=np.float32) / 8.0)
    return np.float32(loss), np.float32(invCR)


def kernel(x, W1, b1, W2, b2, W3, b3, W4, b4, W5, b5, W6, b6, W7, b7):
    global _PROGRAM, _LAST_RESULTS
    from concourse.bass_utils import run_bass_kernel_spmd

    x = np.ascontiguousarray(np.asarray(x, np.float32))
    Wd = dict(W1=np.asarray(W1), W2=np.asarray(W2), W3=np.asarray(W3),
              W4=np.asarray(W4), W5=np.asarray(W5), W6=np.asarray(W6),
              W7=np.asarray(W7), b7=np.asarray(b7))
    for i, bb in enumerate([b1, b2, b3, b4, b5, b6], 1):
        Wd[f"b{i}"] = np.asarray(bb)
    c16, c32 = _pack_consts(Wd)

    if _PROGRAM is None:
        _PROGRAM = _build_program()
    nc = _PROGRAM

    in_maps = []
    for core in range(8):
        xf, mk = _build_xfeat(x[core])
        in_maps.append(dict(xfeat=xf, maskf=mk, c16=c16, c32=c32))

    res = run_bass_kernel_spmd(nc, in_maps, core_ids=list(range(8)),
                               trace=_TRACE, **_TRACE_KW)
    _LAST_RESULTS = res

    ky, kx = np.meshgrid(np.arange(DH), np.arange(DW), indexing="ij")
    tg = 4 * ky + kx
    blk = tg // 16
    row = tg % 16
    D_all = np.zeros((8, 3, DH, DW), np.float32)
    for core in range(8):
        es = res.results[core]["estore"].reshape(NBLK, 16, NL)
        for g in range(3):
            lane = g * 32 + (ky % 32)
            e = es[blk, row, lane].astype(np.float32)
            xc = x[core, g, 3:3 + DH, 3:3 + DW]
            D_all[core, g] = xc - e
    return _finalize_outputs(D_all)


# revision 10
# speedup vs baseline: 1.4442x; 1.4442x over previous
"""Trainium2 Bass kernel for nn_Codec (autoregressive raster-scan codec).

Wavefront decomposition: pixel (ky,kx) of the 122x122 delta grid is computed
at step t = 4*ky + kx (skew-4 anti-diagonal), a 606-step serial chain with 8
cores x 3 images each (data-parallel over the 24 (b,c) pairs); 96 lanes per
core = 3 images x 32 row-slots (slot = ky mod 32).

v2 redesign (vs the shift-DMA baseline):
  - fp16 operands on the PE (1 cycle/row; fp32 runs 2 passes at half rate and
    doubles the LDWEIGHTS+MATMUL instruction count).
  - The kernel stores e(t) = clip(z7*mask) (the clipped prediction) in a
    32-row SBUF ring (row t%32). Delta features dm = x*gridmask - e split:
    the x part is host-precomputed into the feature stream (24 extra rows);
    the e part enters via 4 ring matmuls (one per row-shift q=0..3, lane
    shifts via rhs/out free-dim offset APs, +3 wrap matmuls) using
    phase-packed negated weights (32 phases, row r of phase p holds the
    weight for e(t-k), k=(p-r) mod 32).
  - z1 (48) and the residual z5 preload (24) live in ONE 72-partition PSUM
    tile, so every preload matmul feeds both in one instruction.
  - b7 enters via a constant ones row appended to h6 (13-row rhs).
  - Tail is 2 DVE ops: t0 = z7*mask; e = clip(t0) written into the ring.
    The final delta dm = x_center - e is computed on the HOST (it has x).
  - No gpsimd DMAs at all; every 16 steps the freshly-written ring half is
    copied to a staging tile (DVE) and DMA'd to DRAM.
"""
import sys

sys.path.insert(0, "/opt/trn_rl_repo")
import numpy as np

R = 3
DH = DW = 122
NSTEP = 4 * (DH - 1) + DW  # 606
NL = 96                    # lanes per core = 3 images x 32 slots
CH = 32                    # steps per x-feature chunk
NCHUNK = (NSTEP + CH - 1) // CH          # 19
TPAD = NCHUNK * CH                       # 608
NBLK = (NSTEP + 15) // 16                # 38 output blocks of 16 steps

# (q, d) pairs for the 24 delta features, with W1/W5 column index.
# features 24..30: dy=3 (q=3), dx=-3..3 ; 31..37: q=2 ; 38..44: q=1 ;
# 45..47: left3 = q=0, d=-3..-1
QD = []
for q in (3, 2, 1):
    for d in range(-3, 4):
        QD.append((q, d, 24 + (3 - q) * 7 + (d + 3)))
for d in (-3, -2, -1):
    QD.append((0, d, 48 + d))
assert len(QD) == 24 and all(24 <= c < 48 for (_, _, c) in QD)

_TRACE = False
_TRACE_KW = {}
_LAST_RESULTS = None

# ---------------------------------------------------------------- consts layout
_C16 = {}
_cc16 = 0


def _span16(name, rows, cols):
    global _cc16
    _C16[name] = (rows, _cc16, cols)
    _cc16 += cols


_span16("wx", 48, 88)

# ring k ranges per shift q (k>=2; k=1 comes fresh from the e tile)
WIN_K = {0: range(2, 4), 1: range(2, 8), 2: range(5, 12), 3: range(9, 16)}

for _q in range(4):
    for _p in range(32):
        _span16(f"wd{_q}_{_p}", 32, 88)
_span16("wf47", 1, 88)
_span16("wf44", 1, 88)
_span16("w2T", 48, 48)
_span16("w3T", 48, 48)
_span16("w4T", 48, 48)
_span16("w5T", 48, 24)
_span16("w6T", 24, 12)
_span16("w7b", 13, 1)
CC16 = _cc16

_C32 = {}
_cc32 = 0


def _span32(name, rows, cols):
    global _cc32
    _C32[name] = (rows, _cc32, cols)
    _cc32 += cols


for _i in range(1, 7):
    _span32(f"b{_i}", 48, 1)
CC32 = _cc32


def _pack_consts(W):
    """Returns (c16 (48, CC16) float16, c32 (48, CC32) float32)."""
    c16 = np.zeros((48, CC16), np.float16)
    c32 = np.zeros((48, CC32), np.float32)

    def put16(name, arr):
        rows, c0, cols = _C16[name]
        assert arr.shape == (rows, cols), (name, arr.shape)
        c16[:rows, c0:c0 + cols] = arr.astype(np.float16)

    W1, W5 = W["W1"], W["W5"]

    # mm_x stationary (48K x 72M): rows 0-23 raw x_nb -> [W1x | W5x],
    # rows 24-47 xm_nb (x*gridmask at QD offsets) -> [+W1 qd col | +W5 qd col]
    wx = np.zeros((48, 88), np.float32)
    wx[0:24, 0:48] = W1[:, 0:24].T
    wx[0:24, 64:88] = W5[:, 0:24].T
    for j, (q, d, col) in enumerate(QD):
        wx[24 + j, 0:48] = W1[:, col]
        wx[24 + j, 64:88] = W5[:, col]
    put16("wx", wx)

    # ring phase weights: ring row r at step t (phase p=t%32) holds e(t-k),
    # k=(p-r) mod 32; e(t-k) with shift q is feature (q, d=4q-k) ->
    # weight = -W1[:, col], -W5[:, col].  Only k in WIN_K[q] contribute.
    colof = {(q, d): c for (q, d, c) in QD}
    for q in range(4):
        for p in range(32):
            wd = np.zeros((32, 88), np.float32)
            for r_ in range(32):
                k = (p - r_) % 32
                if k in WIN_K[q]:
                    col = colof[(q, 4 * q - k)]
                    wd[r_, 0:48] = -W1[:, col]
                    wd[r_, 64:88] = -W5[:, col]
            put16(f"wd{q}_{p}", wd)

    wf47 = np.zeros((1, 88), np.float32)
    wf47[0, 0:48] = -W1[:, 47]
    wf47[0, 64:88] = -W5[:, 47]
    put16("wf47", wf47)
    wf44 = np.zeros((1, 88), np.float32)
    wf44[0, 0:48] = -W1[:, 44]
    wf44[0, 64:88] = -W5[:, 44]
    put16("wf44", wf44)
    put16("w2T", W["W2"].T)
    put16("w3T", W["W3"].T)
    put16("w4T", W["W4"].T)
    put16("w5T", W5.T)
    put16("w6T", W["W6"].T)
    put16("w7b", np.concatenate([W["W7"].T, W["b7"][None, :]], axis=0))

    def put32(name, arr):
        rows, c0, cols = _C32[name]
        assert arr.shape == (rows, cols), (name, arr.shape)
        c32[:rows, c0:c0 + cols] = arr.astype(np.float32)

    for i in range(1, 7):
        b = W[f"b{i}"]
        put32(f"b{i}", np.pad(b[:, None], ((0, 48 - b.shape[0]), (0, 0))))
    return c16, c32


def _build_xfeat(xcore):
    """xcore (3,128,128) -> xf16 (48, TPAD*96) fp16, mk32 (1, TPAD*96) fp32.

    Rows 0-23: raw x neighborhood (matches W1[:, :24] feature order).
    Rows 24-47: x*gridmask at the QD (q,d) offsets (the +x part of the
    delta features; the -e part comes from the on-device ring).
    """
    xf = np.zeros((48, TPAD, NL), np.float16)
    mk = np.zeros((1, TPAD, NL), np.float32)
    ky, kx = np.meshgrid(np.arange(DH), np.arange(DW), indexing="ij")
    tf = (4 * ky + kx).ravel()
    F = np.empty((DH, DW, 48), np.float32)
    for g in range(3):
        img = xcore[g]
        col = (g * 32 + (ky % 32)).ravel()
        for i in range(3):
            for j in range(7):
                F[:, :, 7 * i + j] = img[i:i + DH, j:j + DW]
        for j in range(3):
            F[:, :, 21 + j] = img[3:3 + DH, j:j + DW]
        for j, (q, d, _) in enumerate(QD):
            a, b = ky - q, kx + d
            v = np.zeros((DH, DW), np.float32)
            ok = (a >= 0) & (a < DH) & (b >= 0) & (b < DW)
            am, bm = np.clip(a, 0, DH - 1), np.clip(b, 0, DW - 1)
            v = np.where(ok, img[am + 3, bm + 3], 0.0)
            F[:, :, 24 + j] = v
        xf[:, tf, col] = F.reshape(-1, 48).T.astype(np.float16)
        mk[0, tf, col] = 1.0
    return xf.reshape(48, TPAD * NL), mk.reshape(1, TPAD * NL)


def _g3(ap):
    return ap.rearrange("p (g c) -> p g c", g=3)


def _build_program():
    import concourse.bass as bass  # noqa: F401
    from concourse import bacc
    import concourse.mybir as mybir
    from concourse.tile import TileContext

    F32 = mybir.dt.float32
    F16 = mybir.dt.float16
    AF = mybir.ActivationFunctionType
    OP = mybir.AluOpType

    nc = bacc.Bacc(trn_type="TRN2", num_devices=8)
    xfeat_d = nc.dram_tensor("xfeat", [48, TPAD * NL], F16, kind="ExternalInput")
    mask_d = nc.dram_tensor("maskf", [1, TPAD * NL], F32, kind="ExternalInput")
    c16_d = nc.dram_tensor("c16", [48, CC16], F16, kind="ExternalInput")
    c32_d = nc.dram_tensor("c32", [48, CC32], F32, kind="ExternalInput")
    estore_d = nc.dram_tensor("estore", [NBLK * 16, NL], F16, kind="ExternalOutput")

    with TileContext(nc) as tc:
        with tc.tile_pool(name="wp", bufs=1) as wp, \
             tc.tile_pool(name="chp", bufs=3) as chp, \
             tc.tile_pool(name="mkp", bufs=3) as mkp, \
             tc.tile_pool(name="rp", bufs=1) as rp, \
             tc.tile_pool(name="ep", bufs=2) as ep, \
             tc.tile_pool(name="hp", bufs=2) as hp, \
             tc.tile_pool(name="h6p", bufs=1) as h6p, \
             tc.tile_pool(name="tp", bufs=2) as tp, \
             tc.tile_pool(name="z15p", bufs=2, space="PSUM") as z15p, \
             tc.tile_pool(name="zp", bufs=1, space="PSUM") as zp:

            ct16 = wp.tile([48, CC16], F16)
            ct32 = wp.tile([48, CC32], F32)
            nc.sync.dma_start(out=ct16, in_=c16_d[:, :])
            nc.sync.dma_start(out=ct32, in_=c32_d[:, :])

            def cs16(name, r0=0, rows=None):
                r, c0, cols = _C16[name]
                rr = r if rows is None else rows
                return ct16[r0:r0 + rr, c0:c0 + cols]

            def cs32(name, r0=0, rows=None):
                r, c0, cols = _C32[name]
                rr = r if rows is None else rows
                return ct32[r0:r0 + rr, c0:c0 + cols]

            # e ring: row t%32 holds e(t) = clip(z7*mask), fp16.
            # Written via per-step DMA from the (1,96) e tile (compute
            # engines cannot address partition offsets other than 0/32/64/96).
            ring = rp.tile([32, NL], F16, tag="ring")
            nc.vector.memset(ring[:, :], 0.0)

            # h6x: rows 0-11 = h6 (ACT), row 12 = const ones (b7 injection)
            h6x = h6p.tile([13, NL], F16, tag="h6x")
            nc.vector.memset(h6x[:, :], 1.0)

            chunks = {}

            def get_chunk(c):
                if c not in chunks and c < NCHUNK:
                    ch_t = chp.tile([48, CH * NL], F16, name="ch", tag="ch")
                    mk_t = mkp.tile([1, CH * NL], F32, name="mk", tag="mk")
                    lo, hi = c * CH * NL, (c + 1) * CH * NL
                    nc.sync.dma_start(out=ch_t, in_=xfeat_d[:, lo:hi])
                    nc.sync.dma_start(out=mk_t, in_=mask_d[:, lo:hi])
                    chunks[c] = (ch_t, mk_t)
                return chunks.get(c)

            for t in range(NSTEP):
                c = t // CH
                ch_t, mk_t = get_chunk(c)
                if t % CH == 0:
                    get_chunk(c + 1)  # prefetch next chunk
                off = (t - c * CH) * NL
                xs = ch_t[0:48, off:off + NL]
                maskr = mk_t[0:1, off:off + NL]
                p = t % 32

                ringg = _g3(ring[:, :])

                # ---------------- z1(48)+z5(24) merged PSUM preload ----------
                z15 = z15p.tile([88, NL], F32, tag="z15")
                z15g = _g3(z15[:, :])
                nc.tensor.matmul(z15[:, :], cs16("wx"), xs, start=True,
                                 stop=(t == 0))
                # full-ring matmuls (32-row rhs, quadrant-aligned).  Weights
                # only use rows with k>=2; the e(t-1)->ring DMA is emitted
                # AFTER these in program order, so they depend only on the
                # e(t-2) DMA — a full step of slack, no stall.
                for q in range(4) if t > 0 else []:
                    w = cs16(f"wd{q}_{p}")
                    if q == 0:
                        nc.tensor.matmul(z15[:, :], w, ring[:, :],
                                         start=False, stop=False)
                    else:
                        nc.tensor.matmul(z15g[:, :, q:32], w,
                                         ringg[:, :, 0:32 - q],
                                         start=False, stop=False)
                        nc.tensor.matmul(z15g[:, :, 0:q], w,
                                         ringg[:, :, 32 - q:32],
                                         start=False, stop=False)
                # deferred: e(t-1) -> ring row (t-1)%32 (first reader: t+1)
                if t > 0:
                    nc.sync.dma_start(out=ring[(t - 1) % 32:(t - 1) % 32 + 1, :],
                                      in_=eprev[:, :])
                # previous 16-row block fully in ring: DMA to DRAM
                if t > 0 and t % 16 == 0:
                    blk = t // 16 - 1
                    r0 = (blk % 2) * 16
                    nc.sync.dma_start(out=estore_d[blk * 16:(blk + 1) * 16, :],
                                      in_=ring[r0:r0 + 16, :])
                # fresh k=1 contributions from e(t-1): feature 47 (q=0) and
                # feature 44 (q=1, lane shift 1 + wrap)
                if t > 0:
                    epg = _g3(eprev[:, :])
                    nc.tensor.matmul(z15[:, :], cs16("wf47"), eprev[:, :],
                                     start=False, stop=False)
                    nc.tensor.matmul(z15g[:, :, 1:32], cs16("wf44"),
                                     epg[:, :, 0:31], start=False, stop=False)
                    nc.tensor.matmul(z15g[:, :, 0:1], cs16("wf44"),
                                     epg[:, :, 31:32], start=False, stop=True)

                # ---------------- MLP chain ----------------
                h1 = hp.tile([48, NL], F16, tag="h1")
                nc.scalar.activation(h1[:, :], z15[0:48, :], AF.Lrelu,
                                     bias=cs32("b1"), scale=1.0, alpha=0.01)
                z2 = zp.tile([48, NL], F32, tag="z2")
                nc.tensor.matmul(z2[:, :], cs16("w2T"), h1[:, :], start=True,
                                 stop=True)
                h2 = hp.tile([48, NL], F16, tag="h2")
                nc.scalar.activation(h2[:, :], z2[:, :], AF.Lrelu,
                                     bias=cs32("b2"), scale=1.0, alpha=0.01)
                z3 = zp.tile([48, NL], F32, tag="z3")
                nc.tensor.matmul(z3[:, :], cs16("w3T"), h2[:, :], start=True,
                                 stop=True)
                h3 = hp.tile([48, NL], F16, tag="h3")
                nc.scalar.activation(h3[:, :], z3[:, :], AF.Lrelu,
                                     bias=cs32("b3"), scale=1.0, alpha=0.01)
                z4 = zp.tile([48, NL], F32, tag="z4")
                nc.tensor.matmul(z4[:, :], cs16("w4T"), h3[:, :], start=True,
                                 stop=True)
                h4 = hp.tile([48, NL], F16, tag="h4")
                nc.scalar.activation(h4[:, :], z4[:, :], AF.Lrelu,
                                     bias=cs32("b4"), scale=1.0, alpha=0.01)
                nc.tensor.matmul(z15[64:88, :], cs16("w5T"), h4[:, :],
                                 start=False, stop=True)
                h5 = hp.tile([24, NL], F16, tag="h5")
                nc.scalar.activation(h5[:, :], z15[64:88, :], AF.Lrelu,
                                     bias=cs32("b5", rows=24), scale=1.0,
                                     alpha=0.01)
                z6 = zp.tile([12, NL], F32, tag="z6")
                nc.tensor.matmul(z6[:, :], cs16("w6T"), h5[:, :], start=True,
                                 stop=True)
                nc.scalar.activation(h6x[0:12, :], z6[:, :], AF.Lrelu,
                                     bias=cs32("b6", rows=12), scale=1.0,
                                     alpha=0.01)
                z7 = zp.tile([1, NL], F32, tag="z7")
                nc.tensor.matmul(z7[:, :], cs16("w7b"), h6x[:, :], start=True,
                                 stop=True)

                # ---------------- tail: e = clip(z7*mask) -> ring row --------
                t0 = tp.tile([1, NL], F32, tag="t0")
                nc.vector.tensor_tensor(out=t0[:, :], in0=z7[:, :], in1=maskr,
                                        op=OP.mult)
                ecur = ep.tile([1, NL], F16, tag="e")
                nc.vector.tensor_scalar(out=ecur[:, :], in0=t0[:, :],
                                        scalar1=1.0, scalar2=-1.0,
                                        op0=OP.min, op1=OP.max)
                eprev = ecur

            # drain: final e -> ring, final estore block(s)
            pL = (NSTEP - 1) % 32
            nc.sync.dma_start(out=ring[pL:pL + 1, :], in_=eprev[:, :])
            for blk in range((NSTEP - 1) // 16, NBLK):
                r0 = (blk % 2) * 16
                nc.sync.dma_start(out=estore_d[blk * 16:(blk + 1) * 16, :],
                                  in_=ring[r0:r0 + 16, :])

    nc.finalize()
    return nc


_PROGRAM = None


def _finalize_outputs(D_all):
    """D_all (8,3,122,122) float32 deltas -> (loss, invCR)."""
    b, ch, h, w = 8, 3, 128, 128
    deltas = np.zeros((b, ch, h - 2, w), np.float32)
    deltas[:, :, R:R + DH, R:R + DW] = D_all
    loss = np.sqrt(np.mean(np.square(deltas), dtype=np.float32), dtype=np.float32)
    de = deltas[:, :, R:, R:-R]
    hist, _ = np.histogram(de, bins=256, range=(-1.0, 1.0))
    prob = hist.astype(np.float32) / np.float32(de.size)
    logp = np.zeros_like(prob)
    np.log2(prob, out=logp, where=prob > 0)
    invCR = np.float32(np.sum(-prob * logp, dtype=np.float32) / 8.0)
    return np.float32(loss), np.float32(invCR)


def kernel(x, W1, b1, W2, b2, W3, b3, W4, b4, W5, b5, W6, b6, W7, b7):
    global _PROGRAM, _LAST_RESULTS
    from concourse.bass_utils import run_bass_kernel_spmd

    x = np.ascontiguousarray(np.asarray(x, np.float32))
    Wd = dict(W1=np.asarray(W1), W2=np.asarray(W2), W3=np.asarray(W3),
              W4=np.asarray(W4), W5=np.asarray(W5), W6=np.asarray(W6),
              W7=np.asarray(W7), b7=np.asarray(b7))
    for i, bb in enumerate([b1, b2, b3, b4, b5, b6], 1):
        Wd[f"b{i}"] = np.asarray(bb)
    c16, c32 = _pack_consts(Wd)

    if _PROGRAM is None:
        _PROGRAM = _build_program()
    nc = _PROGRAM

    in_maps = []
    for core in range(8):
        xf, mk = _build_xfeat(x[core])
        in_maps.append(dict(xfeat=xf, maskf=mk, c16=c16, c32=c32))

    res = run_bass_kernel_spmd(nc, in_maps, core_ids=list(range(8)),
                               trace=_TRACE, **_TRACE_KW)
    _LAST_RESULTS = res

    ky, kx = np.meshgrid(np.arange(DH), np.arange(DW), indexing="ij")
    tg = 4 * ky + kx
    blk = tg // 16
    row = tg % 16
    D_all = np.zeros((8, 3, DH, DW), np.float32)
    for core in range(8):
        es = res.results[core]["estore"].reshape(NBLK, 16, NL)
        for g in range(3):
            lane = g * 32 + (ky % 32)
            e = es[blk, row, lane].astype(np.float32)
            xc = x[core, g, 3:3 + DH, 3:3 + DW]
            D_all[core, g] = xc - e
    return _finalize_outputs(D_all)
